# revision 1
# baseline (speedup 1.0000x reference)
"""Chamfer distance (squared L2) kernel for Trainium2, 8 NeuronCores.

Problem: xyz1 [4, 8192, 3], xyz2 [4, 8192, 3], weights1/2 [4, 8192] (fp32).
  dist1[b,n] = min_m ||xyz1[b,n] - xyz2[b,m]||^2   (clamped at 0)
  dist2[b,m] = min_n ||...||^2
  out = 0.5 * (sum(dist1*w1)/sum(w1) + sum(dist2*w2)/sum(w2))

Sharding: 8 cores = 4 batches x 2 halves of xyz2. Each core computes
  - dist1 partial mins of its batch vs its xyz2 half   [8192]
  - dist2 full mins for its xyz2 half vs all xyz1      [4096]
Host combines (min over the two halves for dist1), clamps, and does the
weighted average (min and the max(.,0) clamp commute).

Device kernel: distance tiles are produced by the TensorEngine via the
homogeneous-coordinate trick: D = nu + nv - 2 u.v is ONE matmul with
K=24 rows (triple-bf16-split of coordinates and norms, ~fp32 accuracy).
K<=24 allows 4 concurrent matmuls packed into the 128x128 PE array via
tile_position row groups. Min-reduction runs on the VectorEngine with
tensor_tensor_reduce (min elementwise of two 2-bank PSUM tiles + running
min-reduce), consuming 2 distance elements/cycle/lane.
"""

import numpy as np
import ml_dtypes

B, N, M = 4, 8192, 8192
NCORES = 8
MH = M // 2  # xyz2 half per core

_bf = ml_dtypes.bfloat16


def _split3(x64):
    """Triple bf16 split of a float64 array: x ~= h + m + l (each bf16)."""
    h = x64.astype(_bf)
    r = x64 - h.astype(np.float64)
    m = r.astype(_bf)
    r2 = r - m.astype(np.float64)
    l = r2.astype(_bf)
    return h, m, l


def _forms(pts):
    """pts [n,3] fp32 -> (Wp, Rp) both [128, n] bf16.

    24-row stationary (W) / moving (R) forms such that
      (W[:, i]) . (R[:, j]) = ||p_i||^2 + ||q_j||^2 - 2 p_i.q_j
    up to ~1e-6 absolute. Rows are replicated at partition offsets
    0/32/64/96 for 4-way PE row-group packing.
    """
    n = pts.shape[0]
    p64 = np.asarray(pts, dtype=np.float64)
    h, m, l = _split3(p64)  # [n, 3] each
    nrm = (p64 * p64).sum(1)  # [n] exact-ish fp64
    nh, nm, nl = _split3(nrm)

    def neg2(a):  # -2 * bf16 is exact in bf16
        return (a.astype(np.float32) * -2.0).astype(_bf)

    hT, mT, lT = h.T, m.T, l.T  # [3, n]
    W = np.zeros((24, n), _bf)
    R = np.zeros((24, n), _bf)
    # product groups: hh, hm, mh, hl, lh, mm  (ml/lm/ll dropped, ~2^-24)
    W[0:3], R[0:3] = neg2(hT), hT
    W[3:6], R[3:6] = neg2(hT), mT
    W[6:9], R[6:9] = neg2(mT), hT
    W[9:12], R[9:12] = neg2(hT), lT
    W[12:15], R[12:15] = neg2(lT), hT
    W[15:18], R[15:18] = neg2(mT), mT
    # norm rows: ||p||^2 * 1  and  1 * ||q||^2
    W[18], W[19], W[20] = nh, nm, nl
    R[18:21] = np.ones((3, n), _bf)
    W[21:24] = np.ones((3, n), _bf)
    R[21], R[22], R[23] = nh, nm, nl

    Wp = np.zeros((128, n), _bf)
    Rp = np.zeros((128, n), _bf)
    for g in range(4):
        Wp[32 * g : 32 * g + 24] = W
        Rp[32 * g : 32 * g + 24] = R
    return Wp, Rp


def _build_bass(Np, Mp):
    """Bass program for one core: x1 forms with Np points, x2-half forms
    with Mp points. Outputs d1 [128, Np/128] (min over the Mp points for
    each x1 point) and d2 [128, Mp/128] (min over the Np points)."""
    from contextlib import ExitStack

    import concourse.bacc as bacc
    import concourse.tile as tile
    from concourse import mybir

    bf16, f32 = mybir.dt.bfloat16, mybir.dt.float32
    mn = mybir.AluOpType.min

    nc = bacc.Bacc("TRN2", target_bir_lowering=False, debug=False)
    w1 = nc.dram_tensor("w1", [128, Np], bf16, kind="ExternalInput")
    r1 = nc.dram_tensor("r1", [128, Np], bf16, kind="ExternalInput")
    w2 = nc.dram_tensor("w2", [128, Mp], bf16, kind="ExternalInput")
    r2 = nc.dram_tensor("r2", [128, Mp], bf16, kind="ExternalInput")
    d1 = nc.dram_tensor("d1", [128, Np // 128], f32, kind="ExternalOutput")
    d2 = nc.dram_tensor("d2", [128, Mp // 128], f32, kind="ExternalOutput")

    with tile.TileContext(nc) as tc, ExitStack() as ctx:
        consts = ctx.enter_context(tc.tile_pool(name="consts", bufs=1))
        psum_pool = ctx.enter_context(tc.tile_pool(name="psum", bufs=2, space="PSUM"))

        w1_sb = consts.tile([128, Np], bf16, tag="w1")
        r1_sb = consts.tile([128, Np], bf16, tag="r1")
        w2_sb = consts.tile([128, Mp], bf16, tag="w2")
        r2_sb = consts.tile([128, Mp], bf16, tag="r2")
        d1_sb = consts.tile([128, Np // 128], f32, tag="d1")
        d2_sb = consts.tile([128, Mp // 128], f32, tag="d2")
        # per-round partial-min columns: dir1 has 2 rounds, dir2 has 4
        p1 = [consts.tile([128, Np // 128], f32, name=f"p1_{r}", tag=f"p1_{r}") for r in range(Mp // 2048)]
        p2 = [consts.tile([128, Mp // 128], f32, name=f"p2_{r}", tag=f"p2_{r}") for r in range(Np // 2048)]

        def load(sb, dram, cols):
            step = 2048
            for a in range(0, cols, step):
                b = min(cols, a + step)
                nc.sync.dma_start(out=sb[:, a:b], in_=dram[:, a:b])

        # direction 1 uses w1 (stationary) and r2 (moving) - load them first
        load(w1_sb, w1, Np)
        load(r2_sb, r2, Mp)
        load(w2_sb, w2, Mp)
        load(r1_sb, r1, Np)

        def direction(W_sb, R_sb, w_cols, r_cols, parts):
            n_chunks = w_cols // 128
            rounds = r_cols // 2048  # 4 matmuls of 512 per 4-bank psum tile
            for i in range(n_chunks):
                for r in range(rounds):
                    ps = psum_pool.tile([128, 2048], f32, tag="ps")
                    for k in range(4):
                        j = 4 * r + k
                        nc.tensor.matmul(
                            ps[:, k * 512 : (k + 1) * 512],
                            W_sb[32 * k : 32 * k + 24, i * 128 : (i + 1) * 128],
                            R_sb[32 * k : 32 * k + 24, j * 512 : (j + 1) * 512],
                            start=True,
                            stop=True,
                            tile_position=(32 * k, 0),
                        )
                    nc.vector.tensor_reduce(
                        out=parts[r][:, i : i + 1],
                        in_=ps,
                        axis=mybir.AxisListType.X,
                        op=mn,
                    )

        direction(w1_sb, r2_sb, Np, Mp, p1)
        direction(w2_sb, r1_sb, Mp, Np, p2)

        def fold(parts, d_sb):
            # fold per-round partial mins into d_sb with elementwise TT-min
            cur = parts[0]
            for t in parts[1:-1]:
                nxt = consts.tile(list(cur.shape), f32, name=f"fold_{id(t)}", tag=f"fold_{id(t)}")
                nc.vector.tensor_tensor(out=nxt, in0=cur, in1=t, op=mn)
                cur = nxt
            nc.vector.tensor_tensor(out=d_sb, in0=cur, in1=parts[-1], op=mn)

        fold(p1, d1_sb)
        fold(p2, d2_sb)

        nc.sync.dma_start(out=d1[:, :], in_=d1_sb)
        nc.sync.dma_start(out=d2[:, :], in_=d2_sb)

    nc.finalize()
    return nc


def _core_inputs(xyz1, xyz2):
    """Build the 8 per-core input maps from full inputs."""
    forms1 = [_forms(xyz1[b]) for b in range(B)]
    in_maps = []
    for c in range(NCORES):
        b, h = divmod(c, 2)
        W1p, R1p = forms1[b]
        W2p, R2p = _forms(xyz2[b, h * MH : (h + 1) * MH])
        in_maps.append(
            {
                "w1": np.ascontiguousarray(W1p),
                "r1": np.ascontiguousarray(R1p),
                "w2": np.ascontiguousarray(W2p),
                "r2": np.ascontiguousarray(R2p),
            }
        )
    return in_maps


_NC_CACHE = {}


def _get_nc(Np, Mp):
    key = (Np, Mp)
    if key not in _NC_CACHE:
        _NC_CACHE[key] = _build_bass(Np, Mp)
    return _NC_CACHE[key]


def run_cores(in_maps, Np=N, Mp=MH, trace=False, trace_kwargs=None):
    from concourse.bass_utils import run_bass_kernel_spmd

    nc = _get_nc(Np, Mp)
    return run_bass_kernel_spmd(
        nc,
        in_maps,
        core_ids=list(range(len(in_maps))),
        trace=trace,
        **(trace_kwargs or {}),
    )


def kernel(xyz1, xyz2, weights1, weights2):
    xyz1 = np.asarray(xyz1, dtype=np.float32)
    xyz2 = np.asarray(xyz2, dtype=np.float32)
    weights1 = np.asarray(weights1, dtype=np.float32)
    weights2 = np.asarray(weights2, dtype=np.float32)

    in_maps = _core_inputs(xyz1, xyz2)
    results = run_cores(in_maps).results

    dist1 = np.empty((B, N), np.float64)
    dist2 = np.empty((B, M), np.float64)
    for b in range(B):
        ra, rb = results[2 * b], results[2 * b + 1]
        d1 = np.minimum(ra["d1"], rb["d1"])  # [128, N/128]
        dist1[b] = np.maximum(d1.T.reshape(-1), 0.0)
        dist2[b, :MH] = np.maximum(ra["d2"].T.reshape(-1), 0.0)
        dist2[b, MH:] = np.maximum(rb["d2"].T.reshape(-1), 0.0)

    w1 = weights1.astype(np.float64)
    w2 = weights2.astype(np.float64)
    out = 0.5 * ((dist1 * w1).sum() / w1.sum() + (dist2 * w2).sum() / w2.sum())
    return np.asarray(out, dtype=np.float32)

